# revision 13
# baseline (speedup 1.0000x reference)
"""Trainium2 Bass kernel for nn_LocalFeatureEncoder.

Computes, for B=8 batches on 8 NeuronCores (batch b -> core b):
    g      = concat(shape_code, structure_code, pose_code)      # (B, 128)
    local  = einsum('kfz,bz->bkf', W, g) + bias                 # (B, 24, 64)
    out    = einsum('btk,bkf->btf', lbs_weights, local)         # (B, 32768, 64)

Design (v2 — memory-roofline oriented):
  * Host pre-shuffles lbs into bf16 [128, 8192]: partition (j,k') holds
    lbs[t=(tau*128+i)*4+j, k'] at column tau*128+i (k' zero-padded 24->32).
    This puts the contraction axis on partitions, so the main loop needs
    NO PE transposes and no transpose copies.
  * local is computed on device: flat[i, k*64+f] = sum_z g[z] W[k,f,z] via
    3 matmuls (lhsT = g replicated), then scattered into a block-diagonal
    bf16 rhs bd[128, 256] with 4 tiny SBUF->SBUF DMAs + bias add.
  * Main loop: 64 matmuls [128,128]^T @ [128,256] -> PSUM f32, drained in
    [128,512] pairs by alternating DVE/ACT cast-copies to bf16 staging,
    stored with 8 x 512KB DMAs.
  * bf16 end-to-end (inputs host-cast, output host-upcast): per-core HBM
    traffic ~6.8 MB vs 11.5 MB for f32.
  * Input lbs chunks stream on the sync HWDGE ring; stage-1 constants ride
    the scalar ring so they are not FIFO-blocked behind the bulk load.
"""

import os

import numpy as np
import ml_dtypes

import concourse.bass as bass
import concourse.bacc as bacc
import concourse.tile as tile
from concourse import mybir
from concourse import bass_utils
from contextlib import ExitStack

B, T, K, Z, F = 8, 32768, 24, 128, 64
P = 128
JG = 4                  # t-offsets per partition block
PL = JG * K             # 96 live partitions for lbs/bd (no k padding)
TPT = 128               # t-groups (columns of stationary) per tile
NTILES = T // (JG * TPT)       # 64
NF = JG * F                    # 256 bd/matmul output cols
NCOLS = NTILES * TPT           # 8192 lbs cols
IN_CHUNKS = 4
CHUNK_COLS = NCOLS // IN_CHUNKS    # 2048
TILES_PER_CHUNK = NTILES // IN_CHUNKS  # 16
PAIR = 2                # matmul tiles per PSUM bank drain
SBATCH = 8              # tiles per output store (8*256 cols bf16 = 4KB/part)
NSTORES = NTILES // SBATCH     # 8
KF = K * F              # 1536

_built = {}


def _build(hostbd=False):
    key = ("hostbd" if hostbd else "nc")
    if key in _built:
        return _built[key]

    f32 = mybir.dt.float32
    bf16 = mybir.dt.bfloat16
    nc = bacc.Bacc("TRN2", target_bir_lowering=False, debug=False)

    lbs_d = nc.dram_tensor("lbs", (PL, NCOLS), bf16, kind="ExternalInput")
    if hostbd:
        bd_d = nc.dram_tensor("bd", (PL, NF), bf16, kind="ExternalInput")
    else:
        grep_d = nc.dram_tensor("grep", (P, P), bf16, kind="ExternalInput")
        wt_d = nc.dram_tensor("wt", (P, KF), bf16, kind="ExternalInput")
        biasbd_d = nc.dram_tensor("biasbd", (PL, NF), bf16, kind="ExternalInput")
    out_d = nc.dram_tensor("out", (P, NTILES * NF), bf16, kind="ExternalOutput")

    with tile.TileContext(nc) as tc, ExitStack() as ctx:
        # chunk schedule in tiles: small first chunks let the loop start early
        chunk_tiles = [4, 4, 8, 16, 16, 16]
        const = ctx.enter_context(tc.tile_pool(name="const", bufs=1))
        lbs_pool = ctx.enter_context(
            tc.tile_pool(name="lbs_pool", bufs=len(chunk_tiles))
        )
        if not hostbd:
            psS = ctx.enter_context(
                tc.tile_pool(name="psS", bufs=2, space=bass.MemorySpace.PSUM)
            )
        psW = ctx.enter_context(
            tc.tile_pool(name="psW", bufs=1, space=bass.MemorySpace.PSUM)
        )
        psO = ctx.enter_context(
            tc.tile_pool(name="psO", bufs=3, space=bass.MemorySpace.PSUM)
        )
        stag_pool = ctx.enter_context(tc.tile_pool(name="stag_pool", bufs=3))

        # ---- PE pre-warm: ~8 x 512-col dummy matmuls keep the PE busy for
        # ~3.4us from t~6.7us so the HAM un-throttles the clock (1.2->2.4GHz)
        # right as the first lbs chunk lands ----
        wz = const.tile([P, 512], bf16)
        nc.vector.memset(wz[:], 0.0)
        pw = psW.tile([P, 512], f32)
        NWARM = 8
        for i in range(NWARM):
            nc.tensor.matmul(
                pw[:], wz[:, 0:P], wz[:],
                start=(i == 0), stop=(i == NWARM - 1),
            )

        # ---- stage-1 constants FIRST on the sync ring: they are small and
        # complete in ~1.5us; lbs queues behind them FIFO ----
        if hostbd:
            bd = const.tile([PL, NF], bf16)
            nc.sync.dma_start(bd[:], bd_d.ap())
        else:
            wt_sb = const.tile([P, KF], bf16)
            nc.sync.dma_start(wt_sb[:], wt_d.ap())
            grep_sb = const.tile([P, P], bf16)
            nc.sync.dma_start(grep_sb[:], grep_d.ap())
            biasbd_sb = const.tile([PL, NF], bf16)
            nc.sync.dma_start(biasbd_sb[:], biasbd_d.ap())

        # ---- bulk lbs stream on the sync ring ----
        # tile_of[ti] -> (sbuf tile, col offset within it)
        tile_of = {}
        lbs_sb = []
        t0i = 0
        for nt in chunk_tiles:
            t = lbs_pool.tile([PL, nt * TPT], bf16)
            nc.sync.dma_start(
                t[:], lbs_d.ap()[:, t0i * TPT:(t0i + nt) * TPT]
            )
            lbs_sb.append(t)
            for i in range(nt):
                tile_of[t0i + i] = (t, i * TPT)
            t0i += nt

        if not hostbd:
            # ---- stage 1: flat[i, k*64+f] = sum_z g[z] W[k,f,z] (rows identical) ----
            bdt = const.tile([PL, NF], bf16)
            nc.vector.memset(bdt[:], 0.0)

            flat_sb = const.tile([P, KF], bf16)
            for n in range(3):
                fp = psS.tile([P, 512], f32, tag="s1")
                nc.tensor.matmul(
                    fp[:], grep_sb[:], wt_sb[:, n * 512:(n + 1) * 512],
                    start=True, stop=True,
                )
                if n == 1:
                    nc.scalar.copy(flat_sb[:, n * 512:(n + 1) * 512], fp[:])
                else:
                    nc.vector.tensor_copy(flat_sb[:, n * 512:(n + 1) * 512], fp[:])

            # scatter row 0 of flat into the diagonal blocks of bdt; the
            # scalar ring (Q10) is empty and ACT is otherwise idle here
            for j in range(JG):
                nc.scalar.dma_start(
                    bdt[j * K:(j + 1) * K, j * F:(j + 1) * F], flat_sb[0:1, :]
                )
            bd = const.tile([PL, NF], bf16)
            nc.vector.tensor_add(bd[:], bdt[:], biasbd_sb[:])

        # ---- main loop: 64 matmuls, drained in 2-bank quads, 8 store batches ----
        QUAD = 4
        for s in range(NSTORES):
            stag = stag_pool.tile([P, SBATCH * NF], bf16)
            for q in range(SBATCH // QUAD):
                op = psO.tile([P, QUAD * NF], f32)
                for h in range(QUAD):
                    ti = s * SBATCH + q * QUAD + h
                    lt, col = tile_of[ti]
                    nc.tensor.matmul(
                        op[:, h * NF:(h + 1) * NF],
                        lt[:, col:col + TPT],
                        bd[:],
                        start=True, stop=True,
                    )
                dst = stag[:, q * QUAD * NF:(q + 1) * QUAD * NF]
                if (s * 2 + q) % 2 == 0:
                    nc.vector.tensor_copy(dst, op[:])
                else:
                    nc.scalar.copy(dst, op[:])
            # early stores ride the scalar ring ONLY (the sync ring still
            # carries the input stream; a store FIFO'd behind it would stall
            # stag reuse); once the input has drained, alternate rings so the
            # final stores overlap across both queues
            seng = nc.sync if (s >= 5 and s % 2 == 1) else nc.scalar
            seng.dma_start(
                out_d.ap()[:, s * SBATCH * NF:(s + 1) * SBATCH * NF], stag[:]
            )

    nc.compile()
    _built[key] = nc
    return nc


def make_in_maps(inputs, hostbd=False):
    bf16 = ml_dtypes.bfloat16
    g_full = np.concatenate(
        [inputs["shape_code"], inputs["structure_code"], inputs["pose_code"]],
        axis=-1,
    ).astype(np.float32)  # (8, 128)
    # wt[z, k*64+f] = W[k, f, z]
    wt = np.ascontiguousarray(
        inputs["W"].astype(np.float32).transpose(2, 0, 1).reshape(P, KF)
    ).astype(bf16)
    # biasbd: block-diagonal bias, k' padded to 32
    bias = inputs["bias"].astype(np.float32)
    biasbd = np.zeros((JG, K, NF), dtype=np.float32)
    for j in range(JG):
        biasbd[j, :, j * F:(j + 1) * F] = bias

    lbs = inputs["lbs_weights"].astype(np.float32)
    in_maps = []
    for b in range(B):
        # lbs4[j*24+k, tau*128+i] = lbs[b, (tau*128+i)*4+j, k]
        lb = lbs[b].reshape(NCOLS, JG, K).transpose(1, 2, 0)  # (JG, K, 8192)
        m = {"lbs": np.ascontiguousarray(lb.reshape(PL, NCOLS)).astype(bf16)}
        if hostbd:
            # bd = blockdiag(local^T + bias^T), local = einsum('kfz,z->kf')
            local = np.einsum(
                "kfz,z->kf", inputs["W"].astype(np.float32), g_full[b]
            ) + bias
            bdh = np.zeros((JG, K, NF), dtype=np.float32)
            for j in range(JG):
                bdh[j, :, j * F:(j + 1) * F] = local
            m["bd"] = bdh.reshape(PL, NF).astype(bf16)
        else:
            m["grep"] = np.ascontiguousarray(
                np.broadcast_to(g_full[b][:, None], (P, P))
            ).astype(bf16)
            m["wt"] = wt
            m["biasbd"] = biasbd.reshape(PL, NF).astype(bf16)
        in_maps.append(m)
    return in_maps


LAST_RESULT = None


def kernel(**inputs) -> np.ndarray:
    global LAST_RESULT
    hostbd = os.environ.get("LFE_HOSTBD", "0") == "1"
    nc = _build(hostbd)
    in_maps = make_in_maps(inputs, hostbd)
    res = bass_utils.run_bass_kernel_spmd(
        nc,
        in_maps,
        core_ids=list(range(B)),
        trace=os.environ.get("LFE_TRACE", "0") == "1",
    )
    LAST_RESULT = res
    outs = []
    for b in range(B):
        o = np.asarray(res.results[b]["out"]).astype(np.float32)
        # out_d[p, tau*256 + j*64 + f] = out[(tau*128+p)*4+j, f]
        o = o.reshape(P, NTILES, JG, F).transpose(1, 0, 2, 3).reshape(T, F)
        outs.append(o)
    return np.stack(outs, axis=0)


if __name__ == "__main__":
    rng = np.random.default_rng(0)
    inputs = {
        "shape_code": rng.standard_normal((B, 64), dtype=np.float32),
        "structure_code": rng.standard_normal((B, 32), dtype=np.float32),
        "pose_code": rng.standard_normal((B, 32), dtype=np.float32),
        "lbs_weights": rng.random((B, T, K), dtype=np.float32),
        "W": rng.standard_normal((K, F, Z), dtype=np.float32),
        "bias": rng.standard_normal((K, F), dtype=np.float32),
    }
    out = kernel(**inputs)
    g = np.concatenate(
        [inputs["shape_code"], inputs["structure_code"], inputs["pose_code"]], -1
    )
    local = np.einsum("kfz,bz->bkf", inputs["W"], g) + inputs["bias"][None]
    ref = np.einsum("btk,bkf->btf", inputs["lbs_weights"], local)
    err = np.abs(out - ref).max() / np.abs(ref).max()
    print("rel err:", err)
